# revision 18
# baseline (speedup 1.0000x reference)
"""Bidirectional leaky-ESN (B=8,T=2048,D=64,H=1024,O=16) on 8 TRN2 NeuronCores.

Strategy
--------
The recurrence  h_t = 0.1 h_{t-1} + 0.9 tanh(u_proj_t + h_{t-1} W^T)  is a
contraction (leak 0.9, spectral radius 0.9; measured decay ~0.56/step), so
time can be chunked with a short washout: each of the 2 directions x 8
batches is split into C=64 chunks of L=32 steps; every chunk runs
independently from state 0 starting WASH=12 steps early.  Initial-condition
error decays below the bf16 compute floor (~2e-4 vs ~3.5e-3 measured in
simulation against an fp64 oracle).

This turns 2*2048 serial steps into L+WASH=44 steps over 1024 parallel
sequences.  Sharding: cores 0-3 forward direction (batches 2k,2k+1),
cores 4-7 backward - 128 sequences per core = full PE partition width,
single w_out section per core.

With s := h/0.9 the leak folds into W' = 0.9 W and w_out'' = 0.9 w_out:
    s_k = 0.1 s_{k-1} + tanh(u_proj_k + W' s_{k-1}),   h = 0.9 s.
State is kept transposed (H on partitions: 8 tiles [128,128] bf16,
sequences on the free dim).  Per step: 8 u-injection matmuls (K=65,
w_in|w_bias augmented, streamed input prearranged host-side) + 64
W'^T-stationary matmuls accumulate pre-activations into PSUM (8 banks,
one per H-tile); ScalarE tanh -> z (bf16); VectorE computes
s_new = 0.1*s + z (tensor_scalar + tensor_add).  The matmul stream runs
at the issue-rate floor (~56ns per LDWEIGHTS/MATMUL pair, N=128).

States for the L real steps land in a store; readout matmul groups
(q_m = w_out''^T s_m, [16 x 128] PSUM) are interleaved into the loop as
their states become ready, with PSUM->SBUF copies and per-group output
DMAs overlapped.  Host reassembles fwd+bwd+bias into [B,T,O].
"""

import numpy as np
import ml_dtypes

bf16 = ml_dtypes.bfloat16

B, T, D, H, O = 8, 2048, 64, 1024, 16
A = 0.9           # leaky rate
C = 64            # chunks per (batch, direction)
L = T // C        # 32 steps of real output per chunk
WASH = 10         # washout steps
STEPS = L + WASH
NCORES = 8
NI = H // 128     # 8 partition tiles of H
KAUG = D + 1      # 65: input dim + bias indicator row

_cached = {}


def _build_program():
    import concourse.bacc as bacc
    import concourse.mybir as mybir
    from concourse.tile import TileContext

    dt = mybir.dt
    nc = bacc.Bacc(trn_type="TRN2", target_bir_lowering=False, debug=False)

    # wTall[p, j*1024+i] = W'^T[j*128+p, i]: one DMA, 16KB contiguous/partition
    wT_d = nc.dram_tensor("wT", [128, NI * H], dt.bfloat16, kind="ExternalInput").ap()
    winT_d = nc.dram_tensor("winT", [KAUG, H], dt.bfloat16, kind="ExternalInput").ap()
    woutT_d = nc.dram_tensor("woutT", [128, NI * O], dt.bfloat16, kind="ExternalInput").ap()
    vbuf_d = nc.dram_tensor("vbuf", [KAUG, STEPS * 128], dt.bfloat16, kind="ExternalInput").ap()
    qout_d = nc.dram_tensor("qout", [O, L * 128], dt.float32, kind="ExternalOutput").ap()

    with TileContext(nc) as tc:
        _body(tc, mybir, wT_d, winT_d, woutT_d, vbuf_d, qout_d)
    nc.compile()
    return nc


def _body(tc, mybir, wT_d, winT_d, woutT_d, vbuf_d, qout_d):
    dt = mybir.dt
    nc = tc.nc
    Tanh = mybir.ActivationFunctionType.Tanh

    with (
        tc.tile_pool(name="const", bufs=1) as constp,
        tc.tile_pool(name="state", bufs=4) as statep,
        tc.tile_pool(name="zp", bufs=3) as zp,
        tc.tile_pool(name="tp", bufs=3) as tp,
        tc.tile_pool(name="store", bufs=1) as storep,
        tc.tile_pool(name="stage", bufs=1) as stagep,
        tc.tile_pool(name="pre", bufs=1, space="PSUM") as prep,
    ):
        # ---- prologue: load weights + all per-step inputs ----
        winT_sb = constp.tile([KAUG, H], dt.bfloat16, tag="winT", name="winT")
        nc.sync.dma_start(winT_sb[:], winT_d[:])
        vbuf_sb = constp.tile([KAUG, STEPS * 128], dt.bfloat16, tag="vbuf", name="vbuf")
        nc.sync.dma_start(vbuf_sb[:], vbuf_d[:])
        wT_sb = constp.tile([128, NI * H], dt.bfloat16, tag="wT", name="wT")
        nc.sync.dma_start(wT_sb[:], wT_d[:])
        woutT_sb = constp.tile([128, NI * O], dt.bfloat16, tag="woutT", name="woutT")
        nc.sync.dma_start(woutT_sb[:], woutT_d[:])

        store_sb = [storep.tile([128, L * 128], dt.bfloat16, tag=f"st{i}", name=f"st{i}")
                    for i in range(NI)]
        stage_sb = stagep.tile([O, L * 128], dt.float32, tag="stage", name="stage")

        def readout_group(g):
            """q_m = w_out''^T s_m for slots m in [4g, 4g+4): 32 MMs + copy + DMA."""
            pr = prep.tile([O, 512], dt.float32, tag=f"pre{g % NI}", name=f"pr_{g}")
            for mm in range(4):
                m = g * 4 + mm
                for i in range(NI):
                    nc.tensor.matmul(pr[:, mm * 128:(mm + 1) * 128],
                                     woutT_sb[:, i * O:(i + 1) * O],
                                     store_sb[i][:, m * 128:(m + 1) * 128],
                                     start=(i == 0), stop=(i == NI - 1))
            nc.scalar.copy(stage_sb[:, g * 512:(g + 1) * 512], pr)
            nc.sync.dma_start(qout_d[:, g * 512:(g + 1) * 512],
                              stage_sb[:, g * 512:(g + 1) * 512])

        # ---- serial recurrence, all 128 sequences in lockstep ----
        s_prev = None
        for k in range(STEPS):
            vk = vbuf_sb[:, k * 128:(k + 1) * 128]
            if k >= WASH:
                m = k - WASH
                s_cur = [store_sb[i][:, m * 128:(m + 1) * 128] for i in range(NI)]
            else:
                s_cur = [statep.tile([128, 128], dt.bfloat16, tag=f"s{i}", name=f"s{i}_{k}")
                         for i in range(NI)]
            # u-injection first for ALL banks: these depend only on the
            # prefetched input stream, giving the previous step's tanh/update
            # chain ~450ns of PE work to hide behind
            pres = []
            for i in range(NI):
                pre = prep.tile([128, 128], dt.float32, tag=f"pre{i}", name=f"pre{i}_{k}")
                pres.append(pre)
                nc.tensor.matmul(pre, winT_sb[:, i * 128:(i + 1) * 128], vk,
                                 start=True, stop=(k == 0))
            for i in range(NI):
                pre = pres[i]
                if k > 0:
                    for j in range(NI):
                        nc.tensor.matmul(pre, wT_sb[:, j * H + i * 128:j * H + (i + 1) * 128],
                                         s_prev[j], start=False, stop=(j == NI - 1))
                if k == 0:
                    nc.scalar.activation(s_cur[i], pre, Tanh)
                else:
                    z = zp.tile([128, 128], dt.bfloat16, tag=f"z{i}", name=f"z{i}_{k}")
                    nc.scalar.activation(z, pre, Tanh)
                    # s_new = (s_prev * 0.1) + z
                    t01 = tp.tile([128, 128], dt.bfloat16, tag=f"t{i}", name=f"t{i}_{k}")
                    nc.vector.tensor_scalar_mul(t01, s_prev[i], 0.1)
                    nc.vector.tensor_add(s_cur[i], t01, z)
            s_prev = s_cur
            # interleave readout as soon as a 4-slot group of states is complete
            mdone = k - WASH + 1
            if mdone >= 4 and mdone % 4 == 0:
                readout_group(mdone // 4 - 1)


def _prep_inputs(u, w, w_in, w_bias, w_out):
    """Host-side prep: per-core input maps (bf16 except the f32 output)."""
    WT = np.ascontiguousarray((A * w).T).astype(np.float32)               # [j, i]
    wTall = np.ascontiguousarray(
        WT.reshape(NI, 128, H).transpose(1, 0, 2).reshape(128, NI * H)).astype(bf16)
    winT = np.ascontiguousarray(
        np.concatenate([w_in, w_bias[:, None]], axis=1).T).astype(bf16)   # [65, H]
    in_maps = []
    for core in range(NCORES):
        d = core // 4                       # 0 fwd, 1 bwd
        w2 = (A * w_out[1 + d * H:1 + (d + 1) * H, :]).astype(np.float32)  # [H, O]
        woutT = np.ascontiguousarray(
            w2.reshape(NI, 128, O).transpose(1, 0, 2).reshape(128, NI * O)).astype(bf16)
        v = np.zeros((STEPS, KAUG, 128), np.float32)
        ks = np.arange(STEPS)
        for b_loc in range(2):
            b = 2 * (core % 4) + b_loc
            ud = u[b] if d == 0 else u[b, ::-1]
            for c in range(C):
                ts = c * L - WASH + ks
                valid = ts >= 0
                s_idx = b_loc * C + c
                v[valid, :D, s_idx] = ud[ts[valid]]
                v[valid, D, s_idx] = 1.0
        vbuf = np.ascontiguousarray(
            v.transpose(1, 0, 2).reshape(KAUG, STEPS * 128)).astype(bf16)
        in_maps.append({"wT": wTall, "winT": winT, "woutT": woutT, "vbuf": vbuf})
    return in_maps


def _assemble(results, w_out):
    y = np.zeros((B, T, O), np.float32)
    for core in range(NCORES):
        q = np.asarray(results[core]["qout"], np.float32).reshape(O, L, 128)
        d = core // 4
        for b_loc in range(2):
            b = 2 * (core % 4) + b_loc
            qq = q[:, :, b_loc * C:(b_loc + 1) * C]       # [O, L(m), C(c)]
            tmp = qq.transpose(2, 1, 0).reshape(T, O)     # t = c*L + m
            if d == 0:
                y[b] += tmp
            else:
                y[b, ::-1] += tmp
    y += w_out[0][None, None, :].astype(np.float32)
    return y


def kernel(u, w, w_in, w_bias, w_out):
    from concourse.bass_utils import run_bass_kernel_spmd

    u = np.asarray(u, np.float32)
    w = np.asarray(w, np.float32)
    w_in = np.asarray(w_in, np.float32)
    w_bias = np.asarray(w_bias, np.float32)
    w_out = np.asarray(w_out, np.float32)

    if "nc" not in _cached:
        _cached["nc"] = _build_program()
    nc = _cached["nc"]
    in_maps = _prep_inputs(u, w, w_in, w_bias, w_out)
    res = run_bass_kernel_spmd(nc, in_maps, list(range(NCORES)))
    return _assemble(res.results, w_out)


# revision 19
# speedup vs baseline: 1.2317x; 1.2317x over previous
"""Bidirectional leaky-ESN (B=8,T=2048,D=64,H=1024,O=16) on 8 TRN2 NeuronCores.

Strategy
--------
The recurrence  h_t = 0.1 h_{t-1} + 0.9 tanh(u_proj_t + h_{t-1} W^T)  is a
contraction (leak 0.9, spectral radius 0.9; measured decay ~0.56/step), so
time can be chunked with a short washout: each of the 2 directions x 8
batches is split into C=64 chunks of L=32 steps; every chunk runs
independently from state 0 starting WASH=12 steps early.  Initial-condition
error decays below the bf16 compute floor (~2e-4 vs ~3.5e-3 measured in
simulation against an fp64 oracle).

This turns 2*2048 serial steps into L+WASH=44 steps over 1024 parallel
sequences.  Sharding: cores 0-3 forward direction (batches 2k,2k+1),
cores 4-7 backward - 128 sequences per core = full PE partition width,
single w_out section per core.

With s := h/0.9 the leak folds into W' = 0.9 W and w_out'' = 0.9 w_out:
    s_k = 0.1 s_{k-1} + tanh(u_proj_k + W' s_{k-1}),   h = 0.9 s.
State is kept transposed (H on partitions: 8 tiles [128,128] bf16,
sequences on the free dim).  Per step: 8 u-injection matmuls (K=65,
w_in|w_bias augmented, streamed input prearranged host-side) + 64
W'^T-stationary matmuls accumulate pre-activations into PSUM (8 banks,
one per H-tile); ScalarE tanh -> z (bf16); VectorE computes
s_new = 0.1*s + z (tensor_scalar + tensor_add).  The matmul stream runs
at the issue-rate floor (~56ns per LDWEIGHTS/MATMUL pair, N=128).

States for the L real steps land in a store; readout matmul groups
(q_m = w_out''^T s_m, [16 x 128] PSUM) are interleaved into the loop as
their states become ready, with PSUM->SBUF copies and per-group output
DMAs overlapped.  Host reassembles fwd+bwd+bias into [B,T,O].
"""

import numpy as np
import ml_dtypes

bf16 = ml_dtypes.bfloat16

B, T, D, H, O = 8, 2048, 64, 1024, 16
A = 0.9           # leaky rate
C = 64            # chunks per (batch, direction)
L = T // C        # 32 steps of real output per chunk
WASH = 10         # washout steps
STEPS = L + WASH
NCORES = 8
NI = H // 128     # 8 partition tiles of H
KAUG = D + 1      # 65: input dim + bias indicator row

_cached = {}


def _build_program():
    import concourse.bacc as bacc
    import concourse.mybir as mybir
    from concourse.tile import TileContext

    dt = mybir.dt
    nc = bacc.Bacc(trn_type="TRN2", target_bir_lowering=False, debug=False)

    # wTall[p, j*1024+i] = W'^T[j*128+p, i]: one DMA, 16KB contiguous/partition
    wT_d = nc.dram_tensor("wT", [128, NI * H], dt.bfloat16, kind="ExternalInput").ap()
    winT_d = nc.dram_tensor("winT", [KAUG, H], dt.bfloat16, kind="ExternalInput").ap()
    woutT_d = nc.dram_tensor("woutT", [128, NI * O], dt.bfloat16, kind="ExternalInput").ap()
    vbuf_d = nc.dram_tensor("vbuf", [KAUG, STEPS * 128], dt.bfloat16, kind="ExternalInput").ap()
    qout_d = nc.dram_tensor("qout", [O, L * 128], dt.float32, kind="ExternalOutput").ap()

    with TileContext(nc) as tc:
        _body(tc, mybir, wT_d, winT_d, woutT_d, vbuf_d, qout_d)
    nc.compile()
    return nc


def _body(tc, mybir, wT_d, winT_d, woutT_d, vbuf_d, qout_d):
    dt = mybir.dt
    nc = tc.nc
    Tanh = mybir.ActivationFunctionType.Tanh

    with (
        tc.tile_pool(name="const", bufs=1) as constp,
        tc.tile_pool(name="state", bufs=4) as statep,
        tc.tile_pool(name="zp", bufs=3) as zp,
        tc.tile_pool(name="tp", bufs=3) as tp,
        tc.tile_pool(name="store", bufs=1) as storep,
        tc.tile_pool(name="stage", bufs=1) as stagep,
        tc.tile_pool(name="pre", bufs=1, space="PSUM") as prep,
    ):
        # ---- prologue: load weights + all per-step inputs ----
        winT_sb = constp.tile([KAUG, H], dt.bfloat16, tag="winT", name="winT")
        nc.sync.dma_start(winT_sb[:], winT_d[:])
        vbuf_sb = constp.tile([KAUG, STEPS * 128], dt.bfloat16, tag="vbuf", name="vbuf")
        nc.sync.dma_start(vbuf_sb[:], vbuf_d[:])
        wT_sb = constp.tile([128, NI * H], dt.bfloat16, tag="wT", name="wT")
        nc.sync.dma_start(wT_sb[:], wT_d[:])
        woutT_sb = constp.tile([128, NI * O], dt.bfloat16, tag="woutT", name="woutT")
        nc.sync.dma_start(woutT_sb[:], woutT_d[:])

        store_sb = [storep.tile([128, L * 128], dt.bfloat16, tag=f"st{i}", name=f"st{i}")
                    for i in range(NI)]
        stage_sb = stagep.tile([O, L * 128], dt.float32, tag="stage", name="stage")

        def readout_group(g):
            """q_m = w_out''^T s_m for slots m in [4g, 4g+4): 32 MMs + copy + DMA."""
            pr = prep.tile([O, 512], dt.float32, tag=f"pre{g % NI}", name=f"pr_{g}")
            for mm in range(4):
                m = g * 4 + mm
                for i in range(NI):
                    nc.tensor.matmul(pr[:, mm * 128:(mm + 1) * 128],
                                     woutT_sb[:, i * O:(i + 1) * O],
                                     store_sb[i][:, m * 128:(m + 1) * 128],
                                     start=(i == 0), stop=(i == NI - 1))
            nc.scalar.copy(stage_sb[:, g * 512:(g + 1) * 512], pr)
            nc.sync.dma_start(qout_d[:, g * 512:(g + 1) * 512],
                              stage_sb[:, g * 512:(g + 1) * 512])

        # ---- serial recurrence, all 128 sequences in lockstep ----
        s_prev = None
        for k in range(STEPS):
            vk = vbuf_sb[:, k * 128:(k + 1) * 128]
            if k >= WASH:
                m = k - WASH
                s_cur = [store_sb[i][:, m * 128:(m + 1) * 128] for i in range(NI)]
            else:
                s_cur = [statep.tile([128, 128], dt.bfloat16, tag=f"s{i}", name=f"s{i}_{k}")
                         for i in range(NI)]
            for i in range(NI):
                pre = prep.tile([128, 128], dt.float32, tag=f"pre{i}", name=f"pre{i}_{k}")
                nc.tensor.matmul(pre, winT_sb[:, i * 128:(i + 1) * 128], vk,
                                 start=True, stop=(k == 0))
                if k > 0:
                    for j in range(NI):
                        nc.tensor.matmul(pre, wT_sb[:, j * H + i * 128:j * H + (i + 1) * 128],
                                         s_prev[j], start=False, stop=(j == NI - 1))
                if k == 0:
                    nc.scalar.activation(s_cur[i], pre, Tanh)
                else:
                    z = zp.tile([128, 128], dt.bfloat16, tag=f"z{i}", name=f"z{i}_{k}")
                    nc.scalar.activation(z, pre, Tanh)
                    # s_new = (s_prev * 0.1) + z
                    t01 = tp.tile([128, 128], dt.bfloat16, tag=f"t{i}", name=f"t{i}_{k}")
                    nc.vector.tensor_scalar_mul(t01, s_prev[i], 0.1)
                    nc.vector.tensor_add(s_cur[i], t01, z)
            s_prev = s_cur
            # interleave readout as soon as a 4-slot group of states is complete
            mdone = k - WASH + 1
            if mdone >= 4 and mdone % 4 == 0:
                readout_group(mdone // 4 - 1)


def _prep_inputs(u, w, w_in, w_bias, w_out):
    """Host-side prep: per-core input maps (bf16 except the f32 output)."""
    WT = np.ascontiguousarray((A * w).T).astype(np.float32)               # [j, i]
    wTall = np.ascontiguousarray(
        WT.reshape(NI, 128, H).transpose(1, 0, 2).reshape(128, NI * H)).astype(bf16)
    winT = np.ascontiguousarray(
        np.concatenate([w_in, w_bias[:, None]], axis=1).T).astype(bf16)   # [65, H]
    in_maps = []
    for core in range(NCORES):
        d = core // 4                       # 0 fwd, 1 bwd
        w2 = (A * w_out[1 + d * H:1 + (d + 1) * H, :]).astype(np.float32)  # [H, O]
        woutT = np.ascontiguousarray(
            w2.reshape(NI, 128, O).transpose(1, 0, 2).reshape(128, NI * O)).astype(bf16)
        v = np.zeros((STEPS, KAUG, 128), np.float32)
        ks = np.arange(STEPS)
        for b_loc in range(2):
            b = 2 * (core % 4) + b_loc
            ud = u[b] if d == 0 else u[b, ::-1]
            for c in range(C):
                ts = c * L - WASH + ks
                valid = ts >= 0
                s_idx = b_loc * C + c
                v[valid, :D, s_idx] = ud[ts[valid]]
                v[valid, D, s_idx] = 1.0
        vbuf = np.ascontiguousarray(
            v.transpose(1, 0, 2).reshape(KAUG, STEPS * 128)).astype(bf16)
        in_maps.append({"wT": wTall, "winT": winT, "woutT": woutT, "vbuf": vbuf})
    return in_maps


def _assemble(results, w_out):
    y = np.zeros((B, T, O), np.float32)
    for core in range(NCORES):
        q = np.asarray(results[core]["qout"], np.float32).reshape(O, L, 128)
        d = core // 4
        for b_loc in range(2):
            b = 2 * (core % 4) + b_loc
            qq = q[:, :, b_loc * C:(b_loc + 1) * C]       # [O, L(m), C(c)]
            tmp = qq.transpose(2, 1, 0).reshape(T, O)     # t = c*L + m
            if d == 0:
                y[b] += tmp
            else:
                y[b, ::-1] += tmp
    y += w_out[0][None, None, :].astype(np.float32)
    return y


def kernel(u, w, w_in, w_bias, w_out):
    from concourse.bass_utils import run_bass_kernel_spmd

    u = np.asarray(u, np.float32)
    w = np.asarray(w, np.float32)
    w_in = np.asarray(w_in, np.float32)
    w_bias = np.asarray(w_bias, np.float32)
    w_out = np.asarray(w_out, np.float32)

    if "nc" not in _cached:
        _cached["nc"] = _build_program()
    nc = _cached["nc"]
    in_maps = _prep_inputs(u, w, w_in, w_bias, w_out)
    res = run_bass_kernel_spmd(nc, in_maps, list(range(NCORES)))
    return _assemble(res.results, w_out)
